# revision 58
# baseline (speedup 1.0000x reference)
"""AdaBiRealBasicBlock on 8 TRN2 NeuronCores.

Data-parallel over batch (32 -> 4 images/core), weights replicated.
BN statistics are globally synced with small AllReduces (128ch x
{sum,sumsq} f32 = 1KB each, one per conv half), pipelined so the first
AllReduce of each layer overlaps the second half's matmuls.

Math:
  b = where(w > tau, +1, -1);  alpha = mean|w| per out-channel
  conv(x, alpha*b) = alpha * conv(x, b)
  BN(alpha*c) then sign  ==  Sign(s*c + t) with
      s = gamma*alpha*rsqrt(alpha^2*var_c + eps),  t = beta - s*mean_c

conv1 streams x as two fp16 planes (x_hi = fp16(x), x_lo =
fp16((x-x_hi)*2^12)) against weight planes +-1 and +-2^-12 -- all
exactly representable in fp16, accumulated in fp32 PSUM, so c1 matches
a plain f32 conv to ~1e-7.  conv2 is exact in fp16 (+-1 inputs and
weights).
"""
import sys

if "/opt/trn_rl_repo" not in sys.path:
    sys.path.insert(0, "/opt/trn_rl_repo")

import numpy as np

import concourse.bass as bass
import concourse.bacc as bacc
import concourse.mybir as mybir
from concourse.tile import TileContext
from concourse import bass_utils

F32 = mybir.dt.float32
FP16 = mybir.dt.float16
AF = mybir.ActivationFunctionType
ALU = mybir.AluOpType
AX = mybir.AxisListType

B, C, H, W = 32, 256, 28, 28
NCORES = 8
BL = B // NCORES            # images per core
HP, WP = H + 2, W + 2       # padded 30x30
IMG = HP * WP               # 900
SP = BL * H * W             # 3136 spatial elements per core
KTAPS = 9
KW = C * KTAPS              # 2304 contraction
EPS = 1e-5
NTOT = float(B * H * W)     # global BN count
CHUNKS = [(i, h0) for i in range(BL) for h0 in (0, H // 2)]  # 8 x [14 rows]
CH_R = H // 2               # 14 rows per chunk
CH_N = CH_R * W             # 392

_NC_CACHE = {}
LAST_RESULT = None
USE_COLLECTIVE = True
LDW_OPT = False  # walrus rejects bass-emitted InstLdweights with ldw-opt


def _patch_ldw_opt():
    """walrus is invoked with --enable-ldw-opt=false by default; flipping it
    lets codegen elide/overlap redundant LDWEIGHTS (8 consecutive matmuls
    share each weight here)."""
    if getattr(bass_utils, "_ldw_patched", False):
        return
    orig = bass_utils.run_command

    def patched(cmd, *a, **kw):
        if LDW_OPT and isinstance(cmd, list):
            cmd = ["--enable-ldw-opt=true" if c == "--enable-ldw-opt=false"
                   else c for c in cmd]
        return orig(cmd, *a, **kw)

    bass_utils.run_command = patched
    bass_utils._ldw_patched = True


def _build_nc():
    nc = bacc.Bacc("TRN2", target_bir_lowering=False, debug=False,
                   num_devices=NCORES)

    x_d = nc.declare_dram_parameter("x", [BL, C, H, W], F32, isOutput=False)
    w1t_d = nc.declare_dram_parameter("w1t", [C, KTAPS, C], F32, isOutput=False)
    w2t_d = nc.declare_dram_parameter("w2t", [C, KTAPS, C], F32, isOutput=False)
    w1o_d = nc.declare_dram_parameter("w1o", [C, KW], F32, isOutput=False)
    w2o_d = nc.declare_dram_parameter("w2o", [C, KW], F32, isOutput=False)
    t1r_d = nc.declare_dram_parameter("tau1r", [128, C], F32, isOutput=False)
    t2r_d = nc.declare_dram_parameter("tau2r", [128, C], F32, isOutput=False)
    g1_d = nc.declare_dram_parameter("gamma1", [C], F32, isOutput=False)
    b1_d = nc.declare_dram_parameter("beta1", [C], F32, isOutput=False)
    g2_d = nc.declare_dram_parameter("gamma2", [C], F32, isOutput=False)
    b2_d = nc.declare_dram_parameter("beta2", [C], F32, isOutput=False)
    out_d = nc.declare_dram_parameter("out", [BL, C, H, W], F32, isOutput=True)

    with TileContext(nc) as tc:
        with (
            tc.tile_pool(name="main", bufs=1) as P,
            tc.tile_pool(name="wscratch", bufs=2) as WS,
            tc.tile_pool(name="sqpool", bufs=2) as SQ,
            tc.tile_pool(name="psum", bufs=1, space="PSUM") as PS,
            tc.tile_pool(name="dram", bufs=1, space="DRAM") as DR,
        ):
            # ---- persistent tiles ----
            x_hi = [P.tile([128, BL * IMG], FP16, name=f"x_hi{k}") for k in range(2)]
            x_lo = [P.tile([128, BL * IMG], FP16, name=f"x_lo{k}") for k in range(2)]
            x_res = [P.tile([128, SP], F32, name=f"x_res{k}") for k in range(2)]
            losc = P.tile([128, SP], F32, name="losc")
            y_pad = [P.tile([128, BL * IMG], FP16, name=f"y_pad{k}") for k in range(2)]
            c1 = [P.tile([128, SP], F32, name=f"c1_{k}") for k in range(2)]
            c2 = [P.tile([128, SP], F32, name=f"c2_{k}") for k in range(2)]
            w1b = [P.tile([128, KW], FP16, name=f"w1b{k}") for k in range(2)]
            w1bl = [P.tile([128, KW], FP16, name=f"w1bl{k}") for k in range(2)]
            w2b = [P.tile([128, KW], FP16, name=f"w2b{k}") for k in range(2)]
            taur1 = P.tile([128, C], F32, name="taur1")
            taur2 = P.tile([128, C], F32, name="taur2")
            g1c = P.tile([128, 2], F32, name="g1c")
            b1c = P.tile([128, 2], F32, name="b1c")
            g2c = P.tile([128, 2], F32, name="g2c")
            b2c = P.tile([128, 2], F32, name="b2c")
            asum1 = P.tile([128, 2], F32, name="asum1")
            asum2 = P.tile([128, 2], F32, name="asum2")
            sums1 = P.tile([128, 16], F32, name="sums1")
            ssq1 = P.tile([128, 16], F32, name="ssq1")
            sums2 = P.tile([128, 16], F32, name="sums2")
            ssq2 = P.tile([128, 16], F32, name="ssq2")
            st1 = [P.tile([128, 2], F32, name=f"st1_{a}") for a in range(2)]
            st2 = [P.tile([128, 2], F32, name=f"st2_{a}") for a in range(2)]
            fin1 = P.tile([128, 4], F32, name="fin1")
            fin2 = P.tile([128, 4], F32, name="fin2")
            s1c = P.tile([128, 2], F32, name="s1c")
            t1c = P.tile([128, 2], F32, name="t1c")
            s2c = P.tile([128, 2], F32, name="s2c")
            t2c = P.tile([128, 2], F32, name="t2c")
            fsc = P.tile([128, 24], F32, name="fsc")  # finalize scratch

            cc_in = [DR.tile([128, 2], F32, name=f"cc_in{j}") for j in range(4)]
            cc_out = [DR.tile([128, 2], F32, addr_space="Shared",
                              name=f"cc_out{j}") for j in range(4)]
            ccw_in = DR.tile([128, 1], F32, name="ccw_in")
            ccw_out = DR.tile([128, 1], F32, addr_space="Shared",
                              name="ccw_out")

            xhv = [x_hi[k].rearrange("p (i h w) -> p i h w", i=BL, h=HP, w=WP)
                   for k in range(2)]
            xlv = [x_lo[k].rearrange("p (i h w) -> p i h w", i=BL, h=HP, w=WP)
                   for k in range(2)]
            yv = [y_pad[k].rearrange("p (i h w) -> p i h w", i=BL, h=HP, w=WP)
                  for k in range(2)]

            # ---- critical prologue: what conv1 needs ----
            def borders(v, eng=None):
                eng = eng or nc.vector
                eng.memset(v[:, :, 0, :], 0.0)
                eng.memset(v[:, :, HP - 1, :], 0.0)
                eng.memset(v[:, :, 1:HP - 1, 0], 0.0)
                eng.memset(v[:, :, 1:HP - 1, WP - 1], 0.0)

            for k in range(2):
                # gpsimd is otherwise idle at prologue; keep DVE free for
                # the weight binarize on the critical path
                borders(xhv[k], nc.gpsimd)
                borders(xlv[k], nc.gpsimd)

            nc.sync.dma_start(out=taur1[:, :], in_=t1r_d.ap())

            def binarize_part(wt_d, taur, dest, k, t0, t1):
                nt = t1 - t0
                raw = WS.tile([128, nt * C], F32, tag="wraw",
                              name=f"wraw_{wt_d.name}_{k}_{t0}")
                nc.sync.dma_start(
                    out=raw[:, :],
                    in_=wt_d.ap()[k * 128:(k + 1) * 128, t0:t1].rearrange(
                        "p t o -> p (t o)"))
                tb = taur.unsqueeze(1).broadcast_to([128, nt, C])
                nc.vector.tensor_tensor(
                    out=raw.rearrange("p (t o) -> p t o", t=nt),
                    in0=raw.rearrange("p (t o) -> p t o", t=nt),
                    in1=tb, op=ALU.subtract)
                nc.scalar.activation(out=dest[k][:, t0 * C:t1 * C],
                                     in_=raw[:, :], func=AF.Sign)

            def binarize(wt_d, taur, dest):
                for k in range(2):
                    binarize_part(wt_d, taur, dest, k, 0, KTAPS)

            # first taps of ci0 binarize first so conv1 starts early
            binarize_part(w1t_d, taur1, w1b, 0, 0, 3)
            binarize_part(w1t_d, taur1, w1b, 0, 3, 6)
            binarize_part(w1t_d, taur1, w1b, 0, 6, KTAPS)
            binarize_part(w1t_d, taur1, w1b, 1, 0, KTAPS)

            # x_hi = fp16(x);  x_lo = fp16((x - x_hi) * 2^12)
            # so that x_hi + 2^-12 * x_lo == x to full f32 precision.
            xsrcf = x_d.ap().rearrange("i (k p) h w -> k p i (h w)", k=2)
            HB = BL // 2
            for k in range(2):
                # scalar-queue DMAs (parallel to weight DMAs on sync), in
                # image-pair halves so the first hi cast starts earlier
                for hh in range(2):
                    isl = slice(hh * HB, (hh + 1) * HB)
                    nc.scalar.dma_start(
                        out=x_res[k].rearrange("p (i hw) -> p i hw",
                                               i=BL)[:, isl],
                        in_=xsrcf[k][:, isl])
                xrv = x_res[k].rearrange("p (i h w) -> p i h w", i=BL, h=H, w=W)
                lov = losc.rearrange("p (i h w) -> p i h w", i=BL, h=H, w=W)
                for hh in range(2):
                    isl = slice(hh * HB, (hh + 1) * HB)
                    hi_int = xhv[k][:, isl, 1:HP - 1, 1:WP - 1]
                    lo_int = xlv[k][:, isl, 1:HP - 1, 1:WP - 1]
                    nc.scalar.activation(out=hi_int, in_=xrv[:, isl],
                                         func=AF.Copy)
                    nc.vector.tensor_tensor(out=lov[:, isl], in0=xrv[:, isl],
                                            in1=hi_int, op=ALU.subtract)
                    nc.scalar.activation(out=lo_int, in_=lov[:, isl],
                                         func=AF.Copy, scale=4096.0)
            for k in range(2):
                # lo-plane weights: +-2^-12 (exact in fp16)
                nc.scalar.activation(out=w1bl[k][:, :], in_=w1b[k][:, :],
                                     func=AF.Copy, scale=2.0 ** -12)
            for k in range(2):
                borders(yv[k])

            # ---- conv builders ----
            def conv_co(tag, planes, co, csb, sums, ssq):
                NP = len(planes)
                pss = [PS.tile([128, CH_N], F32, tag=f"ps{ch}",
                               name=f"ps_{tag}_{co}_{ch}")
                       for ch in range(8)]
                # plane-outer / chunk-mid / tap-inner: chunk ch's PSUM group
                # completes at plane NP-1 position ch, so epilogues spread
                # across the last plane instead of bunching at the end; and
                # plane 0's matmuls don't depend on plane 1's input buffer
                # (lets conv2 start before the second y1 half is finalized).
                def epilogue(ch):
                    cs = csb[co][:, ch * CH_N:(ch + 1) * CH_N]
                    sl = co * 8 + ch
                    nc.vector.tensor_scalar(
                        out=cs, in0=pss[ch][:, :], scalar1=0.0,
                        scalar2=0.0, op0=ALU.add, op1=ALU.add,
                        accum_out=sums[:, sl:sl + 1])
                    sq = SQ.tile([128, CH_N], F32, tag="sq",
                                 name=f"sq_{tag}_{co}_{ch}")
                    nc.scalar.activation(
                        out=sq[:, :], in_=cs, func=AF.Square,
                        accum_out=ssq[:, sl:sl + 1])

                def emit(k, t, ch, im, h0):
                    dy, dx = t // 3, t % 3
                    wtile, view = planes[k]
                    wap = wtile[:, t * C + co * 128:t * C + co * 128 + 128]
                    first = (k == 0 and t == 0)
                    last = (k == NP - 1 and t == KTAPS - 1)
                    mov = view[:, im, h0 + dy:h0 + dy + CH_R, dx:dx + W]
                    nc.tensor.matmul(pss[ch][:, :], wap, mov,
                                     start=first, stop=last)

                # non-final planes tap-outer (each tap's weights cover the
                # next binarize part's latency); final plane chunk-outer so
                # PSUM groups complete one-by-one and epilogues spread out
                for k in range(NP - 1):
                    for t in range(KTAPS):
                        for ch, (im, h0) in enumerate(CHUNKS):
                            emit(k, t, ch, im, h0)
                for ch, (im, h0) in enumerate(CHUNKS):
                    for t in range(KTAPS):
                        emit(NP - 1, t, ch, im, h0)
                    epilogue(ch)

            # ---- per-co stats AllReduce ----
            def stats_co(co, sums, ssq, st, ci, co_buf, fin):
                nc.vector.reduce_sum(out=st[:, 0:1],
                                     in_=sums[:, co * 8:(co + 1) * 8],
                                     axis=AX.X)
                nc.vector.reduce_sum(out=st[:, 1:2],
                                     in_=ssq[:, co * 8:(co + 1) * 8],
                                     axis=AX.X)
                nc.gpsimd.dma_start(out=ci[:, :], in_=st[:, :])
                if USE_COLLECTIVE:
                    nc.gpsimd.collective_compute(
                        "AllReduce", ALU.add,
                        replica_groups=[list(range(NCORES))],
                        ins=[ci.opt()], outs=[co_buf.opt()])
                    nc.gpsimd.dma_start(out=fin[:, 2 * co:2 * co + 2],
                                        in_=co_buf[:, :])
                else:
                    nc.vector.tensor_scalar(out=fin[:, 2 * co:2 * co + 2],
                                            in0=st[:, :],
                                            scalar1=float(NCORES),
                                            scalar2=None, op0=ALU.mult)

            # ---- per-co BN affine finalize: s, t columns ----
            def finalize_co(co, fin, asum, gcol, bcol, s_out, t_out, base):
                Ssum = fin[:, 2 * co:2 * co + 1]
                Ssq = fin[:, 2 * co + 1:2 * co + 2]
                mean = fsc[:, base + 0:base + 1]
                msq = fsc[:, base + 1:base + 2]
                var = fsc[:, base + 2:base + 3]
                alpha = fsc[:, base + 3:base + 4]
                u = fsc[:, base + 4:base + 5]
                tmp = fsc[:, base + 5:base + 6]
                so = s_out[:, co:co + 1]
                to = t_out[:, co:co + 1]
                nc.vector.tensor_scalar(out=mean, in0=Ssum, scalar1=1.0 / NTOT,
                                        scalar2=None, op0=ALU.mult)
                nc.vector.tensor_scalar(out=msq, in0=Ssq, scalar1=1.0 / NTOT,
                                        scalar2=None, op0=ALU.mult)
                nc.vector.tensor_scalar(out=alpha, in0=asum[:, co:co + 1],
                                        scalar1=1.0 / KW, scalar2=None,
                                        op0=ALU.mult)
                nc.vector.tensor_tensor(out=var, in0=mean, in1=mean, op=ALU.mult)
                nc.vector.tensor_tensor(out=var, in0=msq, in1=var,
                                        op=ALU.subtract)
                nc.vector.tensor_tensor(out=u, in0=alpha, in1=alpha, op=ALU.mult)
                nc.vector.tensor_tensor(out=u, in0=u, in1=var, op=ALU.mult)
                nc.vector.tensor_scalar(out=u, in0=u, scalar1=EPS, scalar2=None,
                                        op0=ALU.add)
                # rsqrt = sqrt(1/u) + one Newton step
                nc.vector.reciprocal(out=so, in_=u)
                nc.scalar.activation(out=so, in_=so, func=AF.Sqrt)
                nc.vector.tensor_tensor(out=tmp, in0=so, in1=so, op=ALU.mult)
                nc.vector.tensor_tensor(out=tmp, in0=tmp, in1=u, op=ALU.mult)
                nc.vector.tensor_scalar(out=tmp, in0=tmp, scalar1=-0.5,
                                        scalar2=1.5, op0=ALU.mult, op1=ALU.add)
                nc.vector.tensor_tensor(out=so, in0=so, in1=tmp, op=ALU.mult)
                # s = gamma*alpha*r ; t = beta - s*mean
                nc.vector.tensor_tensor(out=so, in0=so, in1=alpha, op=ALU.mult)
                nc.vector.tensor_tensor(out=so, in0=so,
                                        in1=gcol[:, co:co + 1], op=ALU.mult)
                nc.vector.tensor_tensor(out=tmp, in0=so, in1=mean, op=ALU.mult)
                nc.vector.tensor_tensor(out=to, in0=bcol[:, co:co + 1],
                                        in1=tmp, op=ALU.subtract)

            # warm-up collective: absorbs ncfw first-collective latency under
            # conv1-co0's cover so the layer-boundary AllReduce runs at floor
            if USE_COLLECTIVE:
                nc.gpsimd.dma_start(out=ccw_in[:, :], in_=taur1[:, 0:1])
                nc.gpsimd.collective_compute(
                    "AllReduce", ALU.add,
                    replica_groups=[list(range(NCORES))],
                    ins=[ccw_in.opt()], outs=[ccw_out.opt()])

            planes1 = [(w1b[0], xhv[0]), (w1b[1], xhv[1]),
                       (w1bl[0], xlv[0]), (w1bl[1], xlv[1])]
            planes2 = [(w2b[0], yv[0]), (w2b[1], yv[1])]

            # ================= layer 1 =================
            conv_co("c1", planes1, 0, c1, sums1, ssq1)
            stats_co(0, sums1, ssq1, st1[0], cc_in[0], cc_out[0], fin1)
            conv_co("c1", planes1, 1, c1, sums1, ssq1)
            stats_co(1, sums1, ssq1, st1[1], cc_in[1], cc_out[1], fin1)

            # ---- deferred prologue (fills idle engines during conv1) ----
            nc.sync.dma_start(out=taur2[:, :], in_=t2r_d.ap())
            binarize(w2t_d, taur2, w2b)
            for col, src in ((g1c, g1_d), (b1c, b1_d), (g2c, g2_d), (b2c, b2_d)):
                nc.sync.dma_start(out=col[:, :],
                                  in_=src.ap().rearrange("(a p) -> p a", p=128))

            def alpha_sums(wo_d, asum):
                for co in range(2):
                    wa = WS.tile([128, KW], F32, tag="walpha",
                                 name=f"walpha_{wo_d.name}_{co}")
                    nc.scalar.dma_start(out=wa[:, :],
                                        in_=wo_d.ap()[co * 128:(co + 1) * 128])
                    nc.vector.reduce_sum(out=asum[:, co:co + 1], in_=wa[:, :],
                                         axis=AX.X, apply_absolute_value=True)

            alpha_sums(w1o_d, asum1)
            alpha_sums(w2o_d, asum2)

            # y1 = Sign(s1*c1 + t1) -> fp16 into padded buffer (per co, in
            # image-pair halves so conv2's dependent plane starts earlier)
            for co in range(2):
                finalize_co(co, fin1, asum1, g1c, b1c, s1c, t1c, 6 * co)
                src = c1[co].rearrange("p (i h w) -> p i h w", i=BL, h=H, w=W)
                for hh in range(2):
                    isl = slice(hh * (BL // 2), (hh + 1) * (BL // 2))
                    dst = yv[co][:, isl, 1:HP - 1, 1:WP - 1]
                    nc.scalar.activation(out=dst, in_=src[:, isl],
                                         func=AF.Sign,
                                         bias=t1c[:, co:co + 1],
                                         scale=s1c[:, co:co + 1])

            # ================= layer 2 =================
            conv_co("c2", planes2, 0, c2, sums2, ssq2)
            stats_co(0, sums2, ssq2, st2[0], cc_in[2], cc_out[2], fin2)
            conv_co("c2", planes2, 1, c2, sums2, ssq2)
            stats_co(1, sums2, ssq2, st2[1], cc_in[3], cc_out[3], fin2)

            # out = Sign(s2*c2 + t2 + x), pipelined per half-batch so the
            # out DMA of half 0 overlaps the Sign of half 1
            outdst = out_d.ap().rearrange("i (k p) h w -> k p i (h w)", k=2)
            HSP = SP // 2
            for co in range(2):
                finalize_co(co, fin2, asum2, g2c, b2c, s2c, t2c, 12 + 6 * co)
                for hh in range(2):
                    sl = slice(hh * HSP, (hh + 1) * HSP)
                    nc.vector.scalar_tensor_tensor(
                        out=c2[co][:, sl], in0=c2[co][:, sl],
                        scalar=s2c[:, co:co + 1], in1=x_res[co][:, sl],
                        op0=ALU.mult, op1=ALU.add)
                    nc.scalar.activation(out=c2[co][:, sl], in_=c2[co][:, sl],
                                         func=AF.Sign, bias=t2c[:, co:co + 1])
                    # alternate queues so the two halves' transfers overlap
                    deng = nc.sync if hh == 0 else nc.scalar
                    deng.dma_start(
                        out=outdst[co][:, hh * (BL // 2):(hh + 1) * (BL // 2)],
                        in_=c2[co][:, sl].rearrange("p (i hw) -> p i hw",
                                                    i=BL // 2))

    nc.compile()
    return nc


def _get_nc():
    if "nc" not in _NC_CACHE:
        _patch_ldw_opt()
        _NC_CACHE["nc"] = _build_nc()
    return _NC_CACHE["nc"]


def kernel(x, w1, tau1, gamma1, beta1, w2, tau2, gamma2, beta2,
           trace=False, trace_kwargs=None):
    global LAST_RESULT
    f = np.float32
    x = np.ascontiguousarray(np.asarray(x, f))
    w1 = np.asarray(w1, f)
    w2 = np.asarray(w2, f)
    common = {
        "w1t": np.ascontiguousarray(np.transpose(w1, (1, 2, 3, 0))
                                    ).reshape(C, KTAPS, C),
        "w2t": np.ascontiguousarray(np.transpose(w2, (1, 2, 3, 0))
                                    ).reshape(C, KTAPS, C),
        "w1o": np.ascontiguousarray(w1.reshape(C, KW)),
        "w2o": np.ascontiguousarray(w2.reshape(C, KW)),
        "tau1r": np.ascontiguousarray(
            np.broadcast_to(np.asarray(tau1, f).reshape(1, C), (128, C))),
        "tau2r": np.ascontiguousarray(
            np.broadcast_to(np.asarray(tau2, f).reshape(1, C), (128, C))),
        "gamma1": np.ascontiguousarray(np.asarray(gamma1, f).reshape(C)),
        "beta1": np.ascontiguousarray(np.asarray(beta1, f).reshape(C)),
        "gamma2": np.ascontiguousarray(np.asarray(gamma2, f).reshape(C)),
        "beta2": np.ascontiguousarray(np.asarray(beta2, f).reshape(C)),
    }
    in_maps = [
        {"x": np.ascontiguousarray(x[i * BL:(i + 1) * BL]), **common}
        for i in range(NCORES)
    ]
    nc = _get_nc()
    kwargs = {}
    if trace:
        kwargs["trace"] = True
        if trace_kwargs:
            kwargs.update(trace_kwargs)
    res = bass_utils.run_bass_kernel_spmd(nc, in_maps,
                                          core_ids=list(range(NCORES)),
                                          **kwargs)
    LAST_RESULT = res
    return np.concatenate([res.results[i]["out"] for i in range(NCORES)],
                          axis=0)


# revision 59
# speedup vs baseline: 1.0476x; 1.0476x over previous
"""AdaBiRealBasicBlock on 8 TRN2 NeuronCores.

Data-parallel over batch (32 -> 4 images/core), weights replicated.
BN statistics are globally synced with small AllReduces (128ch x
{sum,sumsq} f32 = 1KB each, one per conv half), pipelined so the first
AllReduce of each layer overlaps the second half's matmuls.

Math:
  b = where(w > tau, +1, -1);  alpha = mean|w| per out-channel
  conv(x, alpha*b) = alpha * conv(x, b)
  BN(alpha*c) then sign  ==  Sign(s*c + t) with
      s = gamma*alpha*rsqrt(alpha^2*var_c + eps),  t = beta - s*mean_c

conv1 streams x as two fp16 planes (x_hi = fp16(x), x_lo =
fp16((x-x_hi)*2^12)) against weight planes +-1 and +-2^-12 -- all
exactly representable in fp16, accumulated in fp32 PSUM, so c1 matches
a plain f32 conv to ~1e-7.  conv2 is exact in fp16 (+-1 inputs and
weights).
"""
import sys

if "/opt/trn_rl_repo" not in sys.path:
    sys.path.insert(0, "/opt/trn_rl_repo")

import numpy as np

import concourse.bass as bass
import concourse.bacc as bacc
import concourse.mybir as mybir
from concourse.tile import TileContext
from concourse import bass_utils

F32 = mybir.dt.float32
FP16 = mybir.dt.float16
AF = mybir.ActivationFunctionType
ALU = mybir.AluOpType
AX = mybir.AxisListType

B, C, H, W = 32, 256, 28, 28
NCORES = 8
BL = B // NCORES            # images per core
HP, WP = H + 2, W + 2       # padded 30x30
IMG = HP * WP               # 900
SP = BL * H * W             # 3136 spatial elements per core
KTAPS = 9
KW = C * KTAPS              # 2304 contraction
EPS = 1e-5
NTOT = float(B * H * W)     # global BN count
CHUNKS = [(i, h0) for i in range(BL) for h0 in (0, H // 2)]  # 8 x [14 rows]
CH_R = H // 2               # 14 rows per chunk
CH_N = CH_R * W             # 392

_NC_CACHE = {}
LAST_RESULT = None
USE_COLLECTIVE = True
LDW_OPT = False  # walrus rejects bass-emitted InstLdweights with ldw-opt


def _patch_ldw_opt():
    """walrus is invoked with --enable-ldw-opt=false by default; flipping it
    lets codegen elide/overlap redundant LDWEIGHTS (8 consecutive matmuls
    share each weight here)."""
    if getattr(bass_utils, "_ldw_patched", False):
        return
    orig = bass_utils.run_command

    def patched(cmd, *a, **kw):
        if LDW_OPT and isinstance(cmd, list):
            cmd = ["--enable-ldw-opt=true" if c == "--enable-ldw-opt=false"
                   else c for c in cmd]
        return orig(cmd, *a, **kw)

    bass_utils.run_command = patched
    bass_utils._ldw_patched = True


def _build_nc():
    nc = bacc.Bacc("TRN2", target_bir_lowering=False, debug=False,
                   num_devices=NCORES)

    x_d = nc.declare_dram_parameter("x", [BL, C, H, W], F32, isOutput=False)
    w1t_d = nc.declare_dram_parameter("w1t", [C, KTAPS, C], F32, isOutput=False)
    w2t_d = nc.declare_dram_parameter("w2t", [C, KTAPS, C], F32, isOutput=False)
    w1o_d = nc.declare_dram_parameter("w1o", [C, KW], F32, isOutput=False)
    w2o_d = nc.declare_dram_parameter("w2o", [C, KW], F32, isOutput=False)
    t1r_d = nc.declare_dram_parameter("tau1r", [128, C], F32, isOutput=False)
    t2r_d = nc.declare_dram_parameter("tau2r", [128, C], F32, isOutput=False)
    g1_d = nc.declare_dram_parameter("gamma1", [C], F32, isOutput=False)
    b1_d = nc.declare_dram_parameter("beta1", [C], F32, isOutput=False)
    g2_d = nc.declare_dram_parameter("gamma2", [C], F32, isOutput=False)
    b2_d = nc.declare_dram_parameter("beta2", [C], F32, isOutput=False)
    out_d = nc.declare_dram_parameter("out", [BL, C, H, W], F32, isOutput=True)

    with TileContext(nc) as tc:
        with (
            tc.tile_pool(name="main", bufs=1) as P,
            tc.tile_pool(name="wscratch", bufs=2) as WS,
            tc.tile_pool(name="sqpool", bufs=2) as SQ,
            tc.tile_pool(name="psum", bufs=1, space="PSUM") as PS,
            tc.tile_pool(name="dram", bufs=1, space="DRAM") as DR,
        ):
            # ---- persistent tiles ----
            x_hi = [P.tile([128, BL * IMG], FP16, name=f"x_hi{k}") for k in range(2)]
            x_lo = [P.tile([128, BL * IMG], FP16, name=f"x_lo{k}") for k in range(2)]
            x_res = [P.tile([128, SP], F32, name=f"x_res{k}") for k in range(2)]
            losc = P.tile([128, SP], F32, name="losc")
            y_pad = [P.tile([128, BL * IMG], FP16, name=f"y_pad{k}") for k in range(2)]
            c1 = [P.tile([128, SP], F32, name=f"c1_{k}") for k in range(2)]
            c2 = [P.tile([128, SP], F32, name=f"c2_{k}") for k in range(2)]
            w1b = [P.tile([128, KW], FP16, name=f"w1b{k}") for k in range(2)]
            w1bl = [P.tile([128, KW], FP16, name=f"w1bl{k}") for k in range(2)]
            w2b = [P.tile([128, KW], FP16, name=f"w2b{k}") for k in range(2)]
            taur1 = P.tile([128, C], F32, name="taur1")
            taur2 = P.tile([128, C], F32, name="taur2")
            g1c = P.tile([128, 2], F32, name="g1c")
            b1c = P.tile([128, 2], F32, name="b1c")
            g2c = P.tile([128, 2], F32, name="g2c")
            b2c = P.tile([128, 2], F32, name="b2c")
            asum1 = P.tile([128, 2], F32, name="asum1")
            asum2 = P.tile([128, 2], F32, name="asum2")
            sums1 = P.tile([128, 16], F32, name="sums1")
            ssq1 = P.tile([128, 16], F32, name="ssq1")
            sums2 = P.tile([128, 16], F32, name="sums2")
            ssq2 = P.tile([128, 16], F32, name="ssq2")
            st1 = [P.tile([128, 2], F32, name=f"st1_{a}") for a in range(2)]
            st2 = [P.tile([128, 2], F32, name=f"st2_{a}") for a in range(2)]
            fin1 = P.tile([128, 4], F32, name="fin1")
            fin2 = P.tile([128, 4], F32, name="fin2")
            s1c = P.tile([128, 2], F32, name="s1c")
            t1c = P.tile([128, 2], F32, name="t1c")
            s2c = P.tile([128, 2], F32, name="s2c")
            t2c = P.tile([128, 2], F32, name="t2c")
            fsc = P.tile([128, 24], F32, name="fsc")  # finalize scratch

            cc_in = [DR.tile([128, 2], F32, name=f"cc_in{j}") for j in range(4)]
            cc_out = [DR.tile([128, 2], F32, addr_space="Shared",
                              name=f"cc_out{j}") for j in range(4)]

            xhv = [x_hi[k].rearrange("p (i h w) -> p i h w", i=BL, h=HP, w=WP)
                   for k in range(2)]
            xlv = [x_lo[k].rearrange("p (i h w) -> p i h w", i=BL, h=HP, w=WP)
                   for k in range(2)]
            yv = [y_pad[k].rearrange("p (i h w) -> p i h w", i=BL, h=HP, w=WP)
                  for k in range(2)]

            # ---- critical prologue: what conv1 needs ----
            def borders(v, eng=None):
                eng = eng or nc.vector
                eng.memset(v[:, :, 0, :], 0.0)
                eng.memset(v[:, :, HP - 1, :], 0.0)
                eng.memset(v[:, :, 1:HP - 1, 0], 0.0)
                eng.memset(v[:, :, 1:HP - 1, WP - 1], 0.0)

            for k in range(2):
                # gpsimd is otherwise idle at prologue; keep DVE free for
                # the weight binarize on the critical path
                borders(xhv[k], nc.gpsimd)
                borders(xlv[k], nc.gpsimd)

            nc.sync.dma_start(out=taur1[:, :], in_=t1r_d.ap())

            def binarize_part(wt_d, taur, dest, k, t0, t1):
                nt = t1 - t0
                raw = WS.tile([128, nt * C], F32, tag="wraw",
                              name=f"wraw_{wt_d.name}_{k}_{t0}")
                nc.sync.dma_start(
                    out=raw[:, :],
                    in_=wt_d.ap()[k * 128:(k + 1) * 128, t0:t1].rearrange(
                        "p t o -> p (t o)"))
                tb = taur.unsqueeze(1).broadcast_to([128, nt, C])
                nc.vector.tensor_tensor(
                    out=raw.rearrange("p (t o) -> p t o", t=nt),
                    in0=raw.rearrange("p (t o) -> p t o", t=nt),
                    in1=tb, op=ALU.subtract)
                nc.scalar.activation(out=dest[k][:, t0 * C:t1 * C],
                                     in_=raw[:, :], func=AF.Sign)

            def binarize(wt_d, taur, dest):
                for k in range(2):
                    binarize_part(wt_d, taur, dest, k, 0, KTAPS)

            # first taps of ci0 binarize first so conv1 starts early
            binarize_part(w1t_d, taur1, w1b, 0, 0, 3)
            binarize_part(w1t_d, taur1, w1b, 0, 3, 6)
            binarize_part(w1t_d, taur1, w1b, 0, 6, KTAPS)
            binarize_part(w1t_d, taur1, w1b, 1, 0, KTAPS)

            # x_hi = fp16(x);  x_lo = fp16((x - x_hi) * 2^12)
            # so that x_hi + 2^-12 * x_lo == x to full f32 precision.
            xsrcf = x_d.ap().rearrange("i (k p) h w -> k p i (h w)", k=2)
            HB = BL // 2
            for k in range(2):
                # scalar-queue DMAs (parallel to weight DMAs on sync), in
                # image-pair halves so the first hi cast starts earlier
                for hh in range(2):
                    isl = slice(hh * HB, (hh + 1) * HB)
                    nc.scalar.dma_start(
                        out=x_res[k].rearrange("p (i hw) -> p i hw",
                                               i=BL)[:, isl],
                        in_=xsrcf[k][:, isl])
                xrv = x_res[k].rearrange("p (i h w) -> p i h w", i=BL, h=H, w=W)
                lov = losc.rearrange("p (i h w) -> p i h w", i=BL, h=H, w=W)
                for hh in range(2):
                    isl = slice(hh * HB, (hh + 1) * HB)
                    hi_int = xhv[k][:, isl, 1:HP - 1, 1:WP - 1]
                    lo_int = xlv[k][:, isl, 1:HP - 1, 1:WP - 1]
                    nc.scalar.activation(out=hi_int, in_=xrv[:, isl],
                                         func=AF.Copy)
                    nc.vector.tensor_tensor(out=lov[:, isl], in0=xrv[:, isl],
                                            in1=hi_int, op=ALU.subtract)
                    nc.scalar.activation(out=lo_int, in_=lov[:, isl],
                                         func=AF.Copy, scale=4096.0)
            for k in range(2):
                # lo-plane weights: +-2^-12 (exact in fp16)
                nc.scalar.activation(out=w1bl[k][:, :], in_=w1b[k][:, :],
                                     func=AF.Copy, scale=2.0 ** -12)
            for k in range(2):
                borders(yv[k])

            # ---- conv builders ----
            def conv_co(tag, planes, co, csb, sums, ssq):
                NP = len(planes)
                pss = [PS.tile([128, CH_N], F32, tag=f"ps{ch}",
                               name=f"ps_{tag}_{co}_{ch}")
                       for ch in range(8)]
                # plane-outer / chunk-mid / tap-inner: chunk ch's PSUM group
                # completes at plane NP-1 position ch, so epilogues spread
                # across the last plane instead of bunching at the end; and
                # plane 0's matmuls don't depend on plane 1's input buffer
                # (lets conv2 start before the second y1 half is finalized).
                def epilogue(ch):
                    cs = csb[co][:, ch * CH_N:(ch + 1) * CH_N]
                    sl = co * 8 + ch
                    nc.vector.tensor_scalar(
                        out=cs, in0=pss[ch][:, :], scalar1=0.0,
                        scalar2=0.0, op0=ALU.add, op1=ALU.add,
                        accum_out=sums[:, sl:sl + 1])
                    sq = SQ.tile([128, CH_N], F32, tag="sq",
                                 name=f"sq_{tag}_{co}_{ch}")
                    nc.scalar.activation(
                        out=sq[:, :], in_=cs, func=AF.Square,
                        accum_out=ssq[:, sl:sl + 1])

                def emit(k, t, ch, im, h0):
                    dy, dx = t // 3, t % 3
                    wtile, view = planes[k]
                    wap = wtile[:, t * C + co * 128:t * C + co * 128 + 128]
                    first = (k == 0 and t == 0)
                    last = (k == NP - 1 and t == KTAPS - 1)
                    mov = view[:, im, h0 + dy:h0 + dy + CH_R, dx:dx + W]
                    nc.tensor.matmul(pss[ch][:, :], wap, mov,
                                     start=first, stop=last)

                # non-final planes tap-outer (each tap's weights cover the
                # next binarize part's latency); final plane chunk-outer so
                # PSUM groups complete one-by-one and epilogues spread out
                for k in range(NP - 1):
                    for t in range(KTAPS):
                        for ch, (im, h0) in enumerate(CHUNKS):
                            emit(k, t, ch, im, h0)
                for ch, (im, h0) in enumerate(CHUNKS):
                    for t in range(KTAPS):
                        emit(NP - 1, t, ch, im, h0)
                    epilogue(ch)

            # ---- per-co stats AllReduce ----
            def stats_co(co, sums, ssq, st, ci, co_buf, fin):
                nc.vector.reduce_sum(out=st[:, 0:1],
                                     in_=sums[:, co * 8:(co + 1) * 8],
                                     axis=AX.X)
                nc.vector.reduce_sum(out=st[:, 1:2],
                                     in_=ssq[:, co * 8:(co + 1) * 8],
                                     axis=AX.X)
                nc.gpsimd.dma_start(out=ci[:, :], in_=st[:, :])
                if USE_COLLECTIVE:
                    nc.gpsimd.collective_compute(
                        "AllReduce", ALU.add,
                        replica_groups=[list(range(NCORES))],
                        ins=[ci.opt()], outs=[co_buf.opt()])
                    nc.gpsimd.dma_start(out=fin[:, 2 * co:2 * co + 2],
                                        in_=co_buf[:, :])
                else:
                    nc.vector.tensor_scalar(out=fin[:, 2 * co:2 * co + 2],
                                            in0=st[:, :],
                                            scalar1=float(NCORES),
                                            scalar2=None, op0=ALU.mult)

            # ---- per-co BN affine finalize: s, t columns ----
            def finalize_co(co, fin, asum, gcol, bcol, s_out, t_out, base):
                Ssum = fin[:, 2 * co:2 * co + 1]
                Ssq = fin[:, 2 * co + 1:2 * co + 2]
                mean = fsc[:, base + 0:base + 1]
                msq = fsc[:, base + 1:base + 2]
                var = fsc[:, base + 2:base + 3]
                alpha = fsc[:, base + 3:base + 4]
                u = fsc[:, base + 4:base + 5]
                tmp = fsc[:, base + 5:base + 6]
                so = s_out[:, co:co + 1]
                to = t_out[:, co:co + 1]
                nc.vector.tensor_scalar(out=mean, in0=Ssum, scalar1=1.0 / NTOT,
                                        scalar2=None, op0=ALU.mult)
                nc.vector.tensor_scalar(out=msq, in0=Ssq, scalar1=1.0 / NTOT,
                                        scalar2=None, op0=ALU.mult)
                nc.vector.tensor_scalar(out=alpha, in0=asum[:, co:co + 1],
                                        scalar1=1.0 / KW, scalar2=None,
                                        op0=ALU.mult)
                nc.vector.tensor_tensor(out=var, in0=mean, in1=mean, op=ALU.mult)
                nc.vector.tensor_tensor(out=var, in0=msq, in1=var,
                                        op=ALU.subtract)
                nc.vector.tensor_tensor(out=u, in0=alpha, in1=alpha, op=ALU.mult)
                nc.vector.tensor_tensor(out=u, in0=u, in1=var, op=ALU.mult)
                nc.vector.tensor_scalar(out=u, in0=u, scalar1=EPS, scalar2=None,
                                        op0=ALU.add)
                # rsqrt = sqrt(1/u) + one Newton step
                nc.vector.reciprocal(out=so, in_=u)
                nc.scalar.activation(out=so, in_=so, func=AF.Sqrt)
                nc.vector.tensor_tensor(out=tmp, in0=so, in1=so, op=ALU.mult)
                nc.vector.tensor_tensor(out=tmp, in0=tmp, in1=u, op=ALU.mult)
                nc.vector.tensor_scalar(out=tmp, in0=tmp, scalar1=-0.5,
                                        scalar2=1.5, op0=ALU.mult, op1=ALU.add)
                nc.vector.tensor_tensor(out=so, in0=so, in1=tmp, op=ALU.mult)
                # s = gamma*alpha*r ; t = beta - s*mean
                nc.vector.tensor_tensor(out=so, in0=so, in1=alpha, op=ALU.mult)
                nc.vector.tensor_tensor(out=so, in0=so,
                                        in1=gcol[:, co:co + 1], op=ALU.mult)
                nc.vector.tensor_tensor(out=tmp, in0=so, in1=mean, op=ALU.mult)
                nc.vector.tensor_tensor(out=to, in0=bcol[:, co:co + 1],
                                        in1=tmp, op=ALU.subtract)

            planes1 = [(w1b[0], xhv[0]), (w1b[1], xhv[1]),
                       (w1bl[0], xlv[0]), (w1bl[1], xlv[1])]
            planes2 = [(w2b[0], yv[0]), (w2b[1], yv[1])]

            # ================= layer 1 =================
            conv_co("c1", planes1, 0, c1, sums1, ssq1)
            stats_co(0, sums1, ssq1, st1[0], cc_in[0], cc_out[0], fin1)
            conv_co("c1", planes1, 1, c1, sums1, ssq1)
            stats_co(1, sums1, ssq1, st1[1], cc_in[1], cc_out[1], fin1)

            # ---- deferred prologue (fills idle engines during conv1) ----
            nc.sync.dma_start(out=taur2[:, :], in_=t2r_d.ap())
            binarize(w2t_d, taur2, w2b)
            for col, src in ((g1c, g1_d), (b1c, b1_d), (g2c, g2_d), (b2c, b2_d)):
                nc.sync.dma_start(out=col[:, :],
                                  in_=src.ap().rearrange("(a p) -> p a", p=128))

            def alpha_sums(wo_d, asum):
                for co in range(2):
                    wa = WS.tile([128, KW], F32, tag="walpha",
                                 name=f"walpha_{wo_d.name}_{co}")
                    nc.scalar.dma_start(out=wa[:, :],
                                        in_=wo_d.ap()[co * 128:(co + 1) * 128])
                    nc.vector.reduce_sum(out=asum[:, co:co + 1], in_=wa[:, :],
                                         axis=AX.X, apply_absolute_value=True)

            alpha_sums(w1o_d, asum1)
            alpha_sums(w2o_d, asum2)

            # y1 = Sign(s1*c1 + t1) -> fp16 into padded buffer (per co, in
            # image-pair halves so conv2's dependent plane starts earlier)
            for co in range(2):
                finalize_co(co, fin1, asum1, g1c, b1c, s1c, t1c, 6 * co)
                src = c1[co].rearrange("p (i h w) -> p i h w", i=BL, h=H, w=W)
                for hh in range(2):
                    isl = slice(hh * (BL // 2), (hh + 1) * (BL // 2))
                    dst = yv[co][:, isl, 1:HP - 1, 1:WP - 1]
                    nc.scalar.activation(out=dst, in_=src[:, isl],
                                         func=AF.Sign,
                                         bias=t1c[:, co:co + 1],
                                         scale=s1c[:, co:co + 1])

            # ================= layer 2 =================
            conv_co("c2", planes2, 0, c2, sums2, ssq2)
            stats_co(0, sums2, ssq2, st2[0], cc_in[2], cc_out[2], fin2)
            conv_co("c2", planes2, 1, c2, sums2, ssq2)
            stats_co(1, sums2, ssq2, st2[1], cc_in[3], cc_out[3], fin2)

            # out = Sign(s2*c2 + t2 + x), pipelined per half-batch so the
            # out DMA of half 0 overlaps the Sign of half 1
            outdst = out_d.ap().rearrange("i (k p) h w -> k p i (h w)", k=2)
            HSP = SP // 2
            for co in range(2):
                finalize_co(co, fin2, asum2, g2c, b2c, s2c, t2c, 12 + 6 * co)
                for hh in range(2):
                    sl = slice(hh * HSP, (hh + 1) * HSP)
                    nc.vector.scalar_tensor_tensor(
                        out=c2[co][:, sl], in0=c2[co][:, sl],
                        scalar=s2c[:, co:co + 1], in1=x_res[co][:, sl],
                        op0=ALU.mult, op1=ALU.add)
                    nc.scalar.activation(out=c2[co][:, sl], in_=c2[co][:, sl],
                                         func=AF.Sign, bias=t2c[:, co:co + 1])
                    # alternate queues so the two halves' transfers overlap
                    deng = nc.sync if hh == 0 else nc.scalar
                    deng.dma_start(
                        out=outdst[co][:, hh * (BL // 2):(hh + 1) * (BL // 2)],
                        in_=c2[co][:, sl].rearrange("p (i hw) -> p i hw",
                                                    i=BL // 2))

    nc.compile()
    return nc


def _get_nc():
    if "nc" not in _NC_CACHE:
        _patch_ldw_opt()
        _NC_CACHE["nc"] = _build_nc()
    return _NC_CACHE["nc"]


def kernel(x, w1, tau1, gamma1, beta1, w2, tau2, gamma2, beta2,
           trace=False, trace_kwargs=None):
    global LAST_RESULT
    f = np.float32
    x = np.ascontiguousarray(np.asarray(x, f))
    w1 = np.asarray(w1, f)
    w2 = np.asarray(w2, f)
    common = {
        "w1t": np.ascontiguousarray(np.transpose(w1, (1, 2, 3, 0))
                                    ).reshape(C, KTAPS, C),
        "w2t": np.ascontiguousarray(np.transpose(w2, (1, 2, 3, 0))
                                    ).reshape(C, KTAPS, C),
        "w1o": np.ascontiguousarray(w1.reshape(C, KW)),
        "w2o": np.ascontiguousarray(w2.reshape(C, KW)),
        "tau1r": np.ascontiguousarray(
            np.broadcast_to(np.asarray(tau1, f).reshape(1, C), (128, C))),
        "tau2r": np.ascontiguousarray(
            np.broadcast_to(np.asarray(tau2, f).reshape(1, C), (128, C))),
        "gamma1": np.ascontiguousarray(np.asarray(gamma1, f).reshape(C)),
        "beta1": np.ascontiguousarray(np.asarray(beta1, f).reshape(C)),
        "gamma2": np.ascontiguousarray(np.asarray(gamma2, f).reshape(C)),
        "beta2": np.ascontiguousarray(np.asarray(beta2, f).reshape(C)),
    }
    in_maps = [
        {"x": np.ascontiguousarray(x[i * BL:(i + 1) * BL]), **common}
        for i in range(NCORES)
    ]
    nc = _get_nc()
    kwargs = {}
    if trace:
        kwargs["trace"] = True
        if trace_kwargs:
            kwargs.update(trace_kwargs)
    res = bass_utils.run_bass_kernel_spmd(nc, in_maps,
                                          core_ids=list(range(NCORES)),
                                          **kwargs)
    LAST_RESULT = res
    return np.concatenate([res.results[i]["out"] for i in range(NCORES)],
                          axis=0)
